# revision 9
# baseline (speedup 1.0000x reference)
"""Trainium2 Bass kernel for nn_AttnDecoder (B=2048,T=200,H=256,FR=80).

Data-parallel over 8 NeuronCores (256 batch rows each). Per core, batch
processed as 2 blocks of 128 partitions.

Layout: batch-on-partition everywhere. Linear layers y = x @ W.T run on the
PE as out[b, o] = lhsT[k=f, m=b].T @ rhs[k=f, n=o] with lhsT = x.T obtained
via PE transpose and rhs = W.T shipped pre-transposed from the host. Biases
are folded in as rank-1 matmuls (ones-column x bias-row) accumulated into
the same PSUM group.

Attention (the large T=200 loop) runs in fp16 on the PE: enc is cast to
fp16 on the host (halves HBM traffic), per-t slices are PE-transposed to
get the h-contraction onto partitions, ep = encT.T @ w1T accumulates in
fp32 PSUM, dp is added on the DVE, tanh on ACT, u = sum_h s*v via DVE
tensor_tensor_reduce accum. Softmax over t uses exp without max subtraction
(|u| <= ||v||_1 ~ 10, safe in fp32). ctx = sum_t e_t * enc_t is computed on
the PE as PSUM accumulation of diag(e_t).T @ enc_t.
"""
import os
import sys
from contextlib import ExitStack

import numpy as np

for _p in ("/opt/trn_rl_repo", "/opt/pypackages"):
    if os.path.isdir(_p) and _p not in sys.path:
        sys.path.append(_p)

import concourse.bass as bass  # noqa: E402
import concourse.tile as tile  # noqa: E402
from concourse import bacc, mybir  # noqa: E402
from concourse._compat import with_exitstack  # noqa: E402

F32 = mybir.dt.float32
F16 = mybir.dt.float16
AF = mybir.ActivationFunctionType
ALU = mybir.AluOpType
AX = mybir.AxisListType

B, T, H, FR, P1, P2 = 2048, 200, 256, 80, 256, 128
OUT = 240
NCORES = 8
BLOC = B // NCORES          # 256 batch rows per core
NB = BLOC // 128            # 2 partition blocks per core
TC = 25                     # t-chunk size
NCH = T // TC               # 8 chunks

# exp() without max subtraction is safe: |u| <= sum|v| ~ 10.
LAST_EXEC_NS = None


def _gru_weights(pfx):
    return [f"{pfx}_wihT", f"{pfx}_whhT", f"{pfx}_bih", f"{pfx}_bhh"]


@with_exitstack
def _kernel_body(ctx: ExitStack, tc_: tile.TileContext, io: dict):
    nc = tc_.nc

    const = ctx.enter_context(tc_.tile_pool(name="const", bufs=1))
    sb = ctx.enter_context(tc_.tile_pool(name="sb", bufs=1))
    slabp = ctx.enter_context(tc_.tile_pool(name="slabp", bufs=2))
    schp = ctx.enter_context(tc_.tile_pool(name="schp", bufs=2))
    enctp = ctx.enter_context(tc_.tile_pool(name="enctp", bufs=4))
    diagp = ctx.enter_context(tc_.tile_pool(name="diagp", bufs=3))
    scrp = ctx.enter_context(tc_.tile_pool(name="scrp", bufs=2))
    # PSUM pools (8 banks):
    tpp = ctx.enter_context(tc_.tile_pool(name="tpp", bufs=2, space="PSUM"))
    epp = ctx.enter_context(tc_.tile_pool(name="epp", bufs=2, space="PSUM"))
    ctxp = ctx.enter_context(tc_.tile_pool(name="ctxp", bufs=1, space="PSUM"))
    gatep = ctx.enter_context(tc_.tile_pool(name="gatep", bufs=1, space="PSUM"))
    miscp = ctx.enter_context(tc_.tile_pool(name="miscp", bufs=1, space="PSUM"))

    # ---------- constants / weights ----------
    def load_w(name, k, o, dt=F32):
        nck = (k + 127) // 128
        tiles = []
        for c in range(nck):
            kc = min(128, k - c * 128)
            t = const.tile([kc, o], dt, tag=f"w_{name}_{c}")
            nc.sync.dma_start(t[:], io[name][c * 128:c * 128 + kc, :])
            tiles.append(t)
        return tiles

    def load_row(name, o):
        t = const.tile([1, o], F32, tag=f"r_{name}")
        nc.sync.dma_start(t[:], io[name])
        return t

    ident32 = const.tile([128, 128], F32)
    nc.sync.dma_start(ident32[:], io["ident32"])
    ident16 = const.tile([128, 128], F16)
    nc.sync.dma_start(ident16[:], io["ident16"])
    ones1 = const.tile([1, 128], F32)
    nc.gpsimd.memset(ones1[:], 1.0)

    fc1wT = load_w("fc1wT", FR, P1)
    fc1b = load_row("fc1_b", P1)
    fc2wT = load_w("fc2wT", P1, P2)
    fc2b = load_row("fc2_b", P2)
    awihT = load_w("attn_wihT", P2, 3 * H)
    awhhT = load_w("attn_whhT", H, 3 * H)
    abih = load_row("attn_bih", 3 * H)
    abhh = load_row("attn_bhh", 3 * H)
    w1T16 = load_w("w1T16", H, H, F16)
    w2T = load_w("w2T", H, H)
    combT = load_w("combT", 2 * H, H)
    combb = load_row("comb_b", H)
    d0wihT = load_w("dec0_wihT", H, 3 * H)
    d0whhT = load_w("dec0_whhT", H, 3 * H)
    d0bih = load_row("dec0_bih", 3 * H)
    d0bhh = load_row("dec0_bhh", 3 * H)
    d1wihT = load_w("dec1_wihT", H, 3 * H)
    d1whhT = load_w("dec1_whhT", H, 3 * H)
    d1bih = load_row("dec1_bih", 3 * H)
    d1bhh = load_row("dec1_bhh", 3 * H)
    outwT = load_w("outwT", H, OUT)
    outb = load_row("out_b", OUT)

    v_rep = const.tile([128, H], F16)
    nc.sync.dma_start(v_rep[:], io["v16"][None, :].to_broadcast((128, H)))

    # ---------- helpers ----------
    def transpose_act(x_ap, k, tag):
        """x [128, k] fp32 -> list of [kc,128] sbuf fp32 tiles (PE transpose)."""
        tiles = []
        for c in range((k + 127) // 128):
            kc = min(128, k - c * 128)
            tp = tpp.tile([128, 128], F32, tag="tp2")
            nc.tensor.transpose(tp[:kc, :], x_ap[:, c * 128:c * 128 + kc],
                                ident32[:])
            st = sb.tile([128, 128], F32, tag=f"T_{tag}_{c}")
            nc.scalar.copy(st[:kc, :], tp[:kc, :])
            tiles.append(st[:kc, :])
        return tiles

    def matmul_lin(psum_ap, xT_tiles, wT_tiles, brows, n_sl=None):
        """psum[128, O] (slice n_sl) = sum_c xT_c.T @ wT_c + ones.T @ brows."""
        o0, o1 = (0, psum_ap.shape[-1]) if n_sl is None else n_sl
        # fp32 moving operand max N=512
        spans = []
        s = o0
        while s < o1:
            e = min(s + 512, o1)
            spans.append((s, e))
            s = e
        for (s, e) in spans:
            pairs = [(xT, wT[:, s:e]) for xT, wT in zip(xT_tiles, wT_tiles)]
            pairs += [(ones1[:], br[:, s:e]) for br in brows]
            for i, (l, r) in enumerate(pairs):
                nc.tensor.matmul(psum_ap[:, s:e], l, r, start=(i == 0),
                                 stop=(i == len(pairs) - 1))

    def gru(xT_tiles, h_nat, hT_tiles, wihT, whhT, bih, bhh, tag):
        """PyTorch GRUCell. Returns new_h sbuf tile [128, H] fp32."""
        gates = gatep.tile([128, 3 * H], F32, tag="gates")
        # RZ slice [0:512]: gi+gh+bih+bhh
        first = True
        for xT, wT in zip(xT_tiles, wihT):
            nc.tensor.matmul(gates[:, 0:512], xT, wT[:, 0:512], start=first,
                             stop=False)
            first = False
        for hT, wT in zip(hT_tiles, whhT):
            nc.tensor.matmul(gates[:, 0:512], hT, wT[:, 0:512], start=False,
                             stop=False)
        nc.tensor.matmul(gates[:, 0:512], ones1[:], bih[:, 0:512], start=False,
                         stop=False)
        nc.tensor.matmul(gates[:, 0:512], ones1[:], bhh[:, 0:512], start=False,
                         stop=True)
        # N-gi slice [512:768]: gi_n + bih_n
        first = True
        for xT, wT in zip(xT_tiles, wihT):
            nc.tensor.matmul(gates[:, 512:768], xT, wT[:, 512:768],
                             start=first, stop=False)
            first = False
        nc.tensor.matmul(gates[:, 512:768], ones1[:], bih[:, 512:768],
                         start=False, stop=True)
        # N-gh: gh_n + bhh_n
        ghn = miscp.tile([128, H], F32, tag="misc")
        first = True
        for hT, wT in zip(hT_tiles, whhT):
            nc.tensor.matmul(ghn[:], hT, wT[:, 512:768], start=first,
                             stop=False)
            first = False
        nc.tensor.matmul(ghn[:], ones1[:], bhh[:, 512:768], start=False,
                         stop=True)

        rz = sb.tile([128, 512], F32, tag=f"rz_{tag}")
        nc.scalar.activation(rz[:], gates[:, 0:512], AF.Sigmoid)
        # n = tanh(gi_n + r * gh_n)
        tmp = sb.tile([128, H], F32, tag=f"rghn_{tag}")
        nc.vector.tensor_mul(tmp[:], rz[:, 0:256], ghn[:])
        nc.vector.tensor_add(gates[:, 512:768], gates[:, 512:768], tmp[:])
        nst = sb.tile([128, H], F32, tag=f"nst_{tag}")
        nc.scalar.activation(nst[:], gates[:, 512:768], AF.Tanh)
        # new_h = nst + z*(h - nst)
        hmz = sb.tile([128, H], F32, tag=f"hmz_{tag}")
        nc.vector.tensor_sub(hmz[:], h_nat, nst[:])
        nc.vector.tensor_mul(hmz[:], rz[:, 256:512], hmz[:])
        newh = sb.tile([128, H], F32, tag=f"newh_{tag}")
        nc.vector.tensor_add(newh[:], nst[:], hmz[:])
        return newh

    # ---------- per batch-block ----------
    for bb in range(NB):
        bsl = slice(bb * 128, (bb + 1) * 128)

        # ===== Phase 1: PreNet + attention GRU + dec_proj =====
        x0 = sb.tile([128, FR], F32, tag="x0")
        nc.sync.dma_start(x0[:], io["inp"][bsl, :])
        x0T = transpose_act(x0[:], FR, "x0")
        pre1p = miscp.tile([128, P1], F32, tag="misc")
        matmul_lin(pre1p[:], x0T, fc1wT, [fc1b])
        pre1 = sb.tile([128, P1], F32, tag="pre1")
        nc.scalar.activation(pre1[:], pre1p[:], AF.Relu)
        pre1T = transpose_act(pre1[:], P1, "pre1")

        pre2p = miscp.tile([128, P2], F32, tag="misc")
        matmul_lin(pre2p[:], pre1T, fc2wT, [fc2b])
        pre2 = sb.tile([128, P2], F32, tag="pre2")
        nc.scalar.activation(pre2[:], pre2p[:], AF.Relu)
        pre2T = transpose_act(pre2[:], P2, "pre2")

        ah = sb.tile([128, H], F32, tag="ah")
        nc.sync.dma_start(ah[:], io["attn_h"][bsl, :])
        ahT = transpose_act(ah[:], H, "ah")
        nah = gru(pre2T, ah[:], ahT, awihT, awhhT, abih, abhh, "agru")
        nc.sync.dma_start(io["o_attn_h"][bsl, :], nah[:])
        attn_out = sb.tile([128, H], F32, tag="attn_out")
        nc.scalar.activation(attn_out[:], nah[:], AF.Relu)
        attn_outT = transpose_act(attn_out[:], H, "attn_out")

        dpp = miscp.tile([128, H], F32, tag="misc")
        matmul_lin(dpp[:], attn_outT, w2T, [])
        # close accumulation group: matmul_lin with no brows leaves stop unset
        # -> handle by marking last matmul stop via explicit finisher below.
        dp = sb.tile([128, H], F32, tag="dp")
        nc.scalar.copy(dp[:], dpp[:])

        # ===== Phase 2: attention over T =====
        u_all = sb.tile([128, T], F32, tag="u_all")
        e_all = sb.tile([128, T], F32, tag="e_all")
        ctx_ps = ctxp.tile([128, H], F32, tag="ctxps")

        for c in range(NCH):
            slab = slabp.tile([128, TC, H], F16, tag="slab")
            nc.sync.dma_start(slab[:], io["enc16"][bsl, c * TC:(c + 1) * TC, :])
            s_ch = schp.tile([128, TC, H], F16, tag="s_ch")
            for tt in range(TC):
                t = c * TC + tt
                tp = tpp.tile([128, 2, 128], F16, tag="tp2")
                nc.tensor.transpose(tp[:, 0, :], slab[:, tt, 0:128], ident16[:])
                nc.tensor.transpose(tp[:, 1, :], slab[:, tt, 128:256],
                                    ident16[:])
                encT = enctp.tile([128, 2, 128], F16, tag="encT")
                if t % 2 == 0:
                    nc.scalar.copy(encT[:], tp[:])
                else:
                    nc.vector.tensor_copy(encT[:], tp[:])
                ep = epp.tile([128, H], F32, tag="ep")
                nc.tensor.matmul(ep[:], encT[:, 0, :], w1T16[0][:, :],
                                 start=True, stop=False)
                nc.tensor.matmul(ep[:], encT[:, 1, :], w1T16[1][:, :],
                                 start=False, stop=True)
                nc.vector.tensor_add(ep[:], ep[:], dp[:])
                nc.scalar.activation(s_ch[:, tt, :], ep[:], AF.Tanh)
                scr = scrp.tile([128, H], F16, tag="scr")
                nc.vector.tensor_mul(scr[:], s_ch[:, tt, :], v_rep[:])
                nc.vector.reduce_sum(u_all[:, t:t + 1], scr[:], axis=AX.X)
            nc.scalar.activation(e_all[:, c * TC:(c + 1) * TC],
                                 u_all[:, c * TC:(c + 1) * TC], AF.Exp)
            for tt in range(TC):
                t = c * TC + tt
                diag = diagp.tile([128, 128], F16, tag="diag")
                nc.scalar.activation(diag[:], ident16[:], AF.Copy,
                                     scale=e_all[:, t:t + 1])
                nc.tensor.matmul(ctx_ps[:], diag[:], slab[:, tt, :],
                                 start=(t == 0), stop=(t == T - 1))

        ssum = sb.tile([128, 1], F32, tag="ssum")
        nc.vector.reduce_sum(ssum[:], e_all[:], axis=AX.X)
        rcp = sb.tile([128, 1], F32, tag="rcp")
        nc.vector.reciprocal(rcp[:], ssum[:])
        a_sb = sb.tile([128, T], F32, tag="a_sb")
        nc.scalar.activation(a_sb[:], e_all[:], AF.Copy, scale=rcp[:])
        nc.sync.dma_start(io["o_a"][bsl, :], a_sb[:])
        ctx_sb = sb.tile([128, H], F32, tag="ctx_sb")
        nc.scalar.activation(ctx_sb[:], ctx_ps[:], AF.Copy, scale=rcp[:])
        ctxT = transpose_act(ctx_sb[:], H, "ctx")

        # ===== Phase 3: decoder =====
        dinp = miscp.tile([128, H], F32, tag="misc")
        matmul_lin(dinp[:], attn_outT + ctxT, combT, [combb])
        dec_in = sb.tile([128, H], F32, tag="dec_in")
        nc.scalar.copy(dec_in[:], dinp[:])
        dec_inT = transpose_act(dec_in[:], H, "dec_in")

        h0 = sb.tile([128, H], F32, tag="h0")
        nc.sync.dma_start(h0[:], io["dec_h0"][bsl, :])
        h0T = transpose_act(h0[:], H, "h0")
        h0n = gru(dec_inT, h0[:], h0T, d0wihT, d0whhT, d0bih, d0bhh, "g0")
        nc.sync.dma_start(io["o_h0"][bsl, :], h0n[:])
        r0 = sb.tile([128, H], F32, tag="r0")
        nc.scalar.activation(r0[:], h0n[:], AF.Relu)
        do1 = sb.tile([128, H], F32, tag="do1")
        nc.vector.tensor_add(do1[:], dec_in[:], r0[:])
        do1T = transpose_act(do1[:], H, "do1")

        h1 = sb.tile([128, H], F32, tag="h1")
        nc.sync.dma_start(h1[:], io["dec_h1"][bsl, :])
        h1T = transpose_act(h1[:], H, "h1")
        h1n = gru(do1T, h1[:], h1T, d1wihT, d1whhT, d1bih, d1bhh, "g1")
        nc.sync.dma_start(io["o_h1"][bsl, :], h1n[:])
        r1 = sb.tile([128, H], F32, tag="r1")
        nc.scalar.activation(r1[:], h1n[:], AF.Relu)
        do2 = sb.tile([128, H], F32, tag="do2")
        nc.vector.tensor_add(do2[:], do1[:], r1[:])
        do2T = transpose_act(do2[:], H, "do2")

        outp = miscp.tile([128, OUT], F32, tag="misc")
        matmul_lin(outp[:], do2T, outwT, [outb])
        # softmax over 240
        nmax = sb.tile([128, 1], F32, tag="nmax")
        nc.vector.tensor_reduce(nmax[:], outp[:], axis=AX.X, op=ALU.max,
                                negate=True)
        eo = sb.tile([128, OUT], F32, tag="eo")
        nc.scalar.activation(eo[:], outp[:], AF.Exp, bias=nmax[:])
        so = sb.tile([128, 1], F32, tag="so")
        nc.vector.reduce_sum(so[:], eo[:], axis=AX.X)
        ro = sb.tile([128, 1], F32, tag="ro")
        nc.vector.reciprocal(ro[:], so[:])
        oo = sb.tile([128, OUT], F32, tag="oo")
        nc.scalar.activation(oo[:], eo[:], AF.Copy, scale=ro[:])
        nc.sync.dma_start(io["o_out"][bsl, :], oo[:])


_NC_CACHE = {}


def _build():
    if "nc" in _NC_CACHE:
        return _NC_CACHE["nc"], _NC_CACHE["io"]
    nc = bacc.Bacc("TRN2", target_bir_lowering=False, debug=False)
    io = {}

    def din(name, shape, dt=F32):
        io[name] = nc.dram_tensor(name, shape, dt, kind="ExternalInput").ap()

    def dout(name, shape, dt=F32):
        io[name] = nc.dram_tensor(name, shape, dt, kind="ExternalOutput").ap()

    din("inp", (BLOC, FR))
    din("attn_h", (BLOC, H))
    din("dec_h0", (BLOC, H))
    din("dec_h1", (BLOC, H))
    din("enc16", (BLOC, T, H), F16)
    din("ident32", (128, 128))
    din("ident16", (128, 128), F16)
    din("v16", (H,), F16)
    din("fc1wT", (FR, P1))
    din("fc1_b", (1, P1))
    din("fc2wT", (P1, P2))
    din("fc2_b", (1, P2))
    din("attn_wihT", (P2, 3 * H))
    din("attn_whhT", (H, 3 * H))
    din("attn_bih", (1, 3 * H))
    din("attn_bhh", (1, 3 * H))
    din("w1T16", (H, H), F16)
    din("w2T", (H, H))
    din("combT", (2 * H, H))
    din("comb_b", (1, H))
    din("dec0_wihT", (H, 3 * H))
    din("dec0_whhT", (H, 3 * H))
    din("dec0_bih", (1, 3 * H))
    din("dec0_bhh", (1, 3 * H))
    din("dec1_wihT", (H, 3 * H))
    din("dec1_whhT", (H, 3 * H))
    din("dec1_bih", (1, 3 * H))
    din("dec1_bhh", (1, 3 * H))
    din("outwT", (H, OUT))
    din("out_b", (1, OUT))
    dout("o_out", (BLOC, OUT))
    dout("o_attn_h", (BLOC, H))
    dout("o_h0", (BLOC, H))
    dout("o_h1", (BLOC, H))
    dout("o_a", (BLOC, T))

    with tile.TileContext(nc) as tc_:
        _kernel_body(tc_, io)
    nc.compile()
    _NC_CACHE["nc"] = nc
    _NC_CACHE["io"] = io
    return nc, io


def _host_pack(inputs):
    """Shared (replicated) weight arrays, host-transposed/cast."""
    f32 = np.float32
    f16 = np.float16

    def T(x):
        return np.ascontiguousarray(np.asarray(x).T.astype(f32))

    def row(x):
        return np.asarray(x, dtype=f32).reshape(1, -1)

    w = {
        "ident32": np.eye(128, dtype=f32),
        "ident16": np.eye(128, dtype=f16),
        "v16": np.asarray(inputs["v"], dtype=f16),
        "fc1wT": T(inputs["fc1_w"]),
        "fc1_b": row(inputs["fc1_b"]),
        "fc2wT": T(inputs["fc2_w"]),
        "fc2_b": row(inputs["fc2_b"]),
        "attn_wihT": T(inputs["attn_wih"]),
        "attn_whhT": T(inputs["attn_whh"]),
        "attn_bih": row(inputs["attn_bih"]),
        "attn_bhh": row(inputs["attn_bhh"]),
        "w1T16": np.ascontiguousarray(
            np.asarray(inputs["w1"]).T).astype(f16),
        "w2T": T(inputs["w2"]),
        "combT": T(inputs["comb_w"]),
        "comb_b": row(inputs["comb_b"]),
        "dec0_wihT": T(inputs["dec0_wih"]),
        "dec0_whhT": T(inputs["dec0_whh"]),
        "dec0_bih": row(inputs["dec0_bih"]),
        "dec0_bhh": row(inputs["dec0_bhh"]),
        "dec1_wihT": T(inputs["dec1_wih"]),
        "dec1_whhT": T(inputs["dec1_whh"]),
        "dec1_bih": row(inputs["dec1_bih"]),
        "dec1_bhh": row(inputs["dec1_bhh"]),
        "outwT": T(inputs["out_w"]),
        "out_b": row(inputs["out_b"]),
    }
    return w


def kernel(trace=False, **inputs):
    global LAST_EXEC_NS
    from concourse import bass_utils

    nc, _ = _build()
    shared = _host_pack(inputs)
    enc16 = np.asarray(inputs["enc"], dtype=np.float16)

    in_maps = []
    for c in range(NCORES):
        sl = slice(c * BLOC, (c + 1) * BLOC)
        m = dict(shared)
        m["inp"] = np.ascontiguousarray(np.asarray(inputs["inp"])[sl]).astype(
            np.float32)
        m["attn_h"] = np.ascontiguousarray(
            np.asarray(inputs["attn_h"])[sl]).astype(np.float32)
        m["dec_h0"] = np.ascontiguousarray(
            np.asarray(inputs["dec_h0"])[sl]).astype(np.float32)
        m["dec_h1"] = np.ascontiguousarray(
            np.asarray(inputs["dec_h1"])[sl]).astype(np.float32)
        m["enc16"] = np.ascontiguousarray(enc16[sl])
        in_maps.append(m)

    res = bass_utils.run_bass_kernel_spmd(
        nc, in_maps, core_ids=list(range(NCORES)), trace=trace)
    LAST_EXEC_NS = res.exec_time_ns

    out = np.concatenate([r["o_out"] for r in res.results], axis=0)
    nah = np.concatenate([r["o_attn_h"] for r in res.results], axis=0)
    h0n = np.concatenate([r["o_h0"] for r in res.results], axis=0)
    h1n = np.concatenate([r["o_h1"] for r in res.results], axis=0)
    a = np.concatenate([r["o_a"] for r in res.results], axis=0)
    return (out, nah, h0n, h1n, a)
